# revision 5
# baseline (speedup 1.0000x reference)
"""AudioOnlySpecAugment — Trainium2 Bass kernel.

Full-input contract: kernel(X[32,1024,1536] f32, lengths[32] i64) -> [32,1024,1536] f32.

Strategy:
  * The SpecAugment masks depend only on a FIXED jax PRNG key (42) and on
    `lengths`, so the tiny tkeep[B,T] / fkeep[B,A] masks are computed on the
    host (exactly mirroring the reference math).
  * The device kernel is pure data-parallel over batch: each of the 8 cores
    gets 4 samples and does a memory-bound broadcast-multiply:
        out[..., :256]  = X[..., :256]                      (video, untouched)
        out[..., 256:]  = X[..., 256:] * tkeep[b,t] * fkeep[b,a]
  * Per sample: one 6.3MB DMA in, one 6.3MB DMA out; time mask applied as a
    per-partition scalar multiply on ACT, freq mask as a tensor-tensor
    multiply on DVE against a partition-broadcast row.
"""

import numpy as np

B, T, D = 32, 1024, 1536
A = 1280          # AUDIO_DIM
V = D - A         # 256 video features, untouched
FREQ_MASK_RATIO = 0.15
TIME_MASK_RATIO = 0.2
NUM_F = 1
NUM_T = 1

NCORES = 8
BPC = B // NCORES  # samples per core
K = T // 128       # 8 time rows per partition


# ----------------------------------------------------------------------------
# Host-side mask generation (bit-exact mirror of the reference)
# ----------------------------------------------------------------------------

def _time_keep(key, lengths, Tn):
    import jax, jax.numpy as jnp
    Bn = lengths.shape[0]
    L = lengths.astype(jnp.int32)
    max_t = jnp.maximum(1, (L.astype(jnp.float32) * TIME_MASK_RATIO).astype(jnp.int32))
    t_idx = jnp.arange(Tn, dtype=jnp.int32)
    keep = jnp.ones((Bn, Tn), dtype=jnp.float32)
    for _ in range(NUM_T):
        key, k1, k2 = jax.random.split(key, 3)
        u1 = jax.random.uniform(k1, (Bn,))
        u2 = jax.random.uniform(k2, (Bn,))
        t = jnp.floor(u1 * max_t.astype(jnp.float32)).astype(jnp.int32) + 1
        t0_max = jnp.maximum(L - t, 0)
        t0 = jnp.floor(u2 * (t0_max + 1).astype(jnp.float32)).astype(jnp.int32)
        m = (t_idx[None, :] >= t0[:, None]) & (t_idx[None, :] < (t0 + t)[:, None])
        m = m & (L[:, None] > 0)
        keep = keep * (1.0 - m.astype(jnp.float32))
    return keep


def _freq_keep(key, Bn, An):
    import jax, jax.numpy as jnp
    max_f = int(An * FREQ_MASK_RATIO)
    keep = jnp.ones((Bn, An), dtype=jnp.float32)
    if FREQ_MASK_RATIO <= 0 or NUM_F <= 0 or max_f <= 0:
        return keep
    f_idx = jnp.arange(An, dtype=jnp.int32)
    for _ in range(NUM_F):
        key, k1, k2 = jax.random.split(key, 3)
        f = jax.random.randint(k1, (Bn,), 1, max_f + 1)
        f0_max = jnp.maximum(An - f, 0)
        f0 = jnp.floor(jax.random.uniform(k2, (Bn,)) * (f0_max + 1).astype(jnp.float32)).astype(jnp.int32)
        m = (f_idx[None, :] >= f0[:, None]) & (f_idx[None, :] < (f0 + f)[:, None])
        m = m & (f[:, None] > 0)
        keep = keep * (1.0 - m.astype(jnp.float32))
    return keep


def _host_masks(lengths):
    # NOTE: must run on the DEFAULT jax device (the neuron backend): the
    # configured PRNG impl is 'rbg', whose bits are backend-dependent, and the
    # reference generates its masks on the default device too.
    # ALSO: lengths must stay a NUMPY array — the reference receives np
    # inputs, so its `max_t = (L*0.2).astype(int32)` truncates on the host;
    # the same cast on the neuron device rounds-to-nearest and shifts the
    # mask span by one row.
    import jax
    key = jax.random.key(42)
    kt, kf = jax.random.split(key)
    tkeep = np.asarray(_time_keep(kt, np.asarray(lengths), T))
    fkeep = np.asarray(_freq_keep(kf, B, A))
    return tkeep, fkeep


# ----------------------------------------------------------------------------
# Bass kernel (per core: X[4,1024,1536], tk[4,1024], fk[4,1280] -> out)
# ----------------------------------------------------------------------------

_BASS_CACHE = {}


def _build_bass():
    import concourse.bass as bass
    import concourse.tile as tile
    from concourse import bacc, mybir

    nc = bacc.Bacc(trn_type="TRN2", debug=False, num_devices=NCORES)
    Xd = nc.dram_tensor("x", [BPC, T, D], mybir.dt.float32, kind="ExternalInput").ap()
    TKd = nc.dram_tensor("tk", [BPC, T], mybir.dt.float32, kind="ExternalInput").ap()
    FKd = nc.dram_tensor("fk", [BPC, A], mybir.dt.float32, kind="ExternalInput").ap()
    Od = nc.dram_tensor("out", [BPC, T, D], mybir.dt.float32, kind="ExternalOutput").ap()

    with tile.TileContext(nc) as tc:
        with tc.tile_pool(name="xpool", bufs=3) as xpool, \
             tc.tile_pool(name="fkpool", bufs=2) as fkpool, \
             tc.tile_pool(name="fixed", bufs=1) as fixed:
            # tk laid out so partition p, col (b, j) = tkeep[b, K*p + j]
            tk_sb = fixed.tile([128, BPC, K], mybir.dt.float32)
            nc.sync.dma_start(out=tk_sb[:], in_=TKd.rearrange("b (p k) -> p b k", k=K))
            # all 4 freq-mask rows on partition 0
            fk_row = fixed.tile([1, BPC, A], mybir.dt.float32)
            nc.sync.dma_start(out=fk_row[:], in_=FKd.rearrange("b a -> (b a)")[None, :].rearrange("o (b a) -> o b a", b=BPC))

            for b in range(BPC):
                fk_bc = fkpool.tile([128, A], mybir.dt.float32)
                nc.gpsimd.partition_broadcast(fk_bc[:], fk_row[0:1, b, :])

                xt = xpool.tile([128, K, D], mybir.dt.float32)
                nc.sync.dma_start(out=xt[:], in_=Xd[b].rearrange("(p k) d -> p k d", k=K))
                for j in range(K):
                    sl = xt[:, j, V:D]
                    # time mask: per-partition scalar multiply on ACT
                    nc.scalar.activation(
                        out=sl, in_=sl,
                        func=mybir.ActivationFunctionType.Copy,
                        scale=tk_sb[:, b, j:j + 1],
                    )
                    # freq mask: elementwise multiply on DVE
                    nc.vector.tensor_mul(out=sl, in0=sl, in1=fk_bc[:])
                nc.sync.dma_start(out=Od[b].rearrange("(p k) d -> p k d", k=K), in_=xt[:])
    nc.compile()
    return nc


def _get_bass():
    if "nc" not in _BASS_CACHE:
        _BASS_CACHE["nc"] = _build_bass()
    return _BASS_CACHE["nc"]


def _run_on_device(X, tkeep, fkeep, trace=False):
    from concourse.bass_utils import run_bass_kernel_spmd

    nc = _get_bass()
    X = np.ascontiguousarray(np.asarray(X, dtype=np.float32))
    in_maps = []
    for i in range(NCORES):
        sl = slice(i * BPC, (i + 1) * BPC)
        in_maps.append({
            "x": np.ascontiguousarray(X[sl]),
            "tk": np.ascontiguousarray(tkeep[sl]),
            "fk": np.ascontiguousarray(fkeep[sl]),
        })
    res = run_bass_kernel_spmd(nc, in_maps, core_ids=list(range(NCORES)), trace=trace)
    out = np.concatenate([r["out"] for r in res.results], axis=0)
    return out, res


def kernel(X, lengths):
    tkeep, fkeep = _host_masks(lengths)
    out, _ = _run_on_device(X, tkeep, fkeep, trace=False)
    return out


# revision 8
# speedup vs baseline: 1.2033x; 1.2033x over previous
"""AudioOnlySpecAugment — Trainium2 Bass kernel.

Full-input contract: kernel(X[32,1024,1536] f32, lengths[32] i64) -> [32,1024,1536] f32.

Strategy:
  * The SpecAugment masks depend only on a FIXED jax PRNG key (42) and on
    `lengths`, so the tiny tkeep[B,T] / fkeep[B,A] masks are computed on the
    host (exactly mirroring the reference math).
  * The device kernel is pure data-parallel over batch: each of the 8 cores
    gets 4 samples and does a memory-bound broadcast-multiply:
        out[..., :256]  = X[..., :256]                      (video, untouched)
        out[..., 256:]  = X[..., 256:] * tkeep[b,t] * fkeep[b,a]
  * Per sample: one 6.3MB DMA in, one 6.3MB DMA out; time mask applied as a
    per-partition scalar multiply on ACT, freq mask as a tensor-tensor
    multiply on DVE against a partition-broadcast row.
"""

import numpy as np

B, T, D = 32, 1024, 1536
A = 1280          # AUDIO_DIM
V = D - A         # 256 video features, untouched
FREQ_MASK_RATIO = 0.15
TIME_MASK_RATIO = 0.2
NUM_F = 1
NUM_T = 1

NCORES = 8
BPC = B // NCORES  # samples per core
K = T // 128       # 8 time rows per partition


# ----------------------------------------------------------------------------
# Host-side mask generation (bit-exact mirror of the reference)
# ----------------------------------------------------------------------------

def _time_keep(key, lengths, Tn):
    import jax, jax.numpy as jnp
    Bn = lengths.shape[0]
    L = lengths.astype(jnp.int32)
    max_t = jnp.maximum(1, (L.astype(jnp.float32) * TIME_MASK_RATIO).astype(jnp.int32))
    t_idx = jnp.arange(Tn, dtype=jnp.int32)
    keep = jnp.ones((Bn, Tn), dtype=jnp.float32)
    for _ in range(NUM_T):
        key, k1, k2 = jax.random.split(key, 3)
        u1 = jax.random.uniform(k1, (Bn,))
        u2 = jax.random.uniform(k2, (Bn,))
        t = jnp.floor(u1 * max_t.astype(jnp.float32)).astype(jnp.int32) + 1
        t0_max = jnp.maximum(L - t, 0)
        t0 = jnp.floor(u2 * (t0_max + 1).astype(jnp.float32)).astype(jnp.int32)
        m = (t_idx[None, :] >= t0[:, None]) & (t_idx[None, :] < (t0 + t)[:, None])
        m = m & (L[:, None] > 0)
        keep = keep * (1.0 - m.astype(jnp.float32))
    return keep


def _freq_keep(key, Bn, An):
    import jax, jax.numpy as jnp
    max_f = int(An * FREQ_MASK_RATIO)
    keep = jnp.ones((Bn, An), dtype=jnp.float32)
    if FREQ_MASK_RATIO <= 0 or NUM_F <= 0 or max_f <= 0:
        return keep
    f_idx = jnp.arange(An, dtype=jnp.int32)
    for _ in range(NUM_F):
        key, k1, k2 = jax.random.split(key, 3)
        f = jax.random.randint(k1, (Bn,), 1, max_f + 1)
        f0_max = jnp.maximum(An - f, 0)
        f0 = jnp.floor(jax.random.uniform(k2, (Bn,)) * (f0_max + 1).astype(jnp.float32)).astype(jnp.int32)
        m = (f_idx[None, :] >= f0[:, None]) & (f_idx[None, :] < (f0 + f)[:, None])
        m = m & (f[:, None] > 0)
        keep = keep * (1.0 - m.astype(jnp.float32))
    return keep


def _host_masks(lengths):
    # NOTE: must run on the DEFAULT jax device (the neuron backend): the
    # configured PRNG impl is 'rbg', whose bits are backend-dependent, and the
    # reference generates its masks on the default device too.
    # ALSO: lengths must stay a NUMPY array — the reference receives np
    # inputs, so its `max_t = (L*0.2).astype(int32)` truncates on the host;
    # the same cast on the neuron device rounds-to-nearest and shifts the
    # mask span by one row.
    import jax
    key = jax.random.key(42)
    kt, kf = jax.random.split(key)
    tkeep = np.asarray(_time_keep(kt, np.asarray(lengths), T))
    fkeep = np.asarray(_freq_keep(kf, B, A))
    return tkeep, fkeep


# ----------------------------------------------------------------------------
# Bass kernel (per core: X[4,1024,1536], tk[4,1024], fk[4,1280] -> out)
# ----------------------------------------------------------------------------

_BASS_CACHE = {}


def _build_bass():
    import concourse.bass as bass
    import concourse.tile as tile
    from concourse import bacc, mybir

    nc = bacc.Bacc(trn_type="TRN2", debug=False, num_devices=NCORES)
    Xd = nc.dram_tensor("x", [BPC, T, D], mybir.dt.float32, kind="ExternalInput").ap()
    TKd = nc.dram_tensor("tk", [BPC, T], mybir.dt.float32, kind="ExternalInput").ap()
    FKd = nc.dram_tensor("fk", [BPC, A], mybir.dt.float32, kind="ExternalInput").ap()
    Od = nc.dram_tensor("out", [BPC, T, D], mybir.dt.float32, kind="ExternalOutput").ap()

    with tile.TileContext(nc) as tc:
        with tc.tile_pool(name="xpool", bufs=3) as xpool, \
             tc.tile_pool(name="fkpool", bufs=2) as fkpool, \
             tc.tile_pool(name="fixed", bufs=1) as fixed:
            # Small mask loads go through gpsimd (SWDGE) so the sync HWDGE
            # ring's first entry is the first big X load.
            # tk laid out so partition p, col (b, j) = tkeep[b, K*p + j]
            tk_sb = fixed.tile([128, BPC, K], mybir.dt.float32)
            nc.gpsimd.dma_start(out=tk_sb[:], in_=TKd.rearrange("b (p k) -> p b k", k=K))
            # all 4 freq-mask rows on partition 0
            fk_row = fixed.tile([1, BPC, A], mybir.dt.float32)
            nc.gpsimd.dma_start(out=fk_row[:], in_=FKd.rearrange("b a -> (b a)")[None, :].rearrange("o (b a) -> o b a", b=BPC))

            for b in range(BPC):
                fk_bc = fkpool.tile([128, A], mybir.dt.float32)
                nc.gpsimd.partition_broadcast(fk_bc[:], fk_row[0:1, b, :])

                xt = xpool.tile([128, K, D], mybir.dt.float32)
                nc.sync.dma_start(out=xt[:], in_=Xd[b].rearrange("(p k) d -> p k d", k=K))
                for j in range(K):
                    sl = xt[:, j, V:D]
                    # time mask: per-partition scalar multiply on ACT
                    nc.scalar.activation(
                        out=sl, in_=sl,
                        func=mybir.ActivationFunctionType.Copy,
                        scale=tk_sb[:, b, j:j + 1],
                    )
                    # freq mask: elementwise multiply on DVE
                    nc.vector.tensor_mul(out=sl, in0=sl, in1=fk_bc[:])
                nc.sync.dma_start(out=Od[b].rearrange("(p k) d -> p k d", k=K), in_=xt[:])
    nc.compile()
    return nc


def _get_bass():
    if "nc" not in _BASS_CACHE:
        _BASS_CACHE["nc"] = _build_bass()
    return _BASS_CACHE["nc"]


def _build_cached_callable(nc):
    """Mirror run_bass_via_pjrt's multi-core branch, but keep the jitted fn
    (no donation) so repeat kernel() calls skip re-lowering/compile."""
    import jax
    from jax.sharding import Mesh, PartitionSpec, NamedSharding
    from jax.experimental.shard_map import shard_map
    from concourse import mybir
    from concourse.bass2jax import _bass_exec_p, install_neuronx_cc_hook, partition_id_tensor

    install_neuronx_cc_hook()
    partition_name = nc.partition_id_tensor.name if nc.partition_id_tensor else None
    in_names, out_names, out_avals = [], [], []
    for alloc in nc.m.functions[0].allocations:
        if not isinstance(alloc, mybir.MemoryLocationSet):
            continue
        name = alloc.memorylocations[0].name
        if alloc.kind == "ExternalInput":
            if name != partition_name:
                in_names.append(name)
        elif alloc.kind == "ExternalOutput":
            out_names.append(name)
            out_avals.append(jax.core.ShapedArray(tuple(alloc.tensor_shape), mybir.dt.np(alloc.dtype)))
    all_in_names = list(in_names) + list(out_names)
    if partition_name is not None:
        all_in_names.append(partition_name)

    def _body(*args):
        operands = list(args)
        if partition_name is not None:
            operands.append(partition_id_tensor())
        return tuple(_bass_exec_p.bind(
            *operands,
            out_avals=tuple(out_avals),
            in_names=tuple(all_in_names),
            out_names=tuple(out_names),
            lowering_input_output_aliases=(),
            sim_require_finite=True,
            sim_require_nnan=True,
            nc=nc,
        ))

    devices = jax.devices()[:NCORES]
    mesh = Mesh(np.asarray(devices), ("core",))
    spec = PartitionSpec("core")
    n_args = len(in_names) + len(out_names)
    f = jax.jit(
        shard_map(_body, mesh=mesh, in_specs=(spec,) * n_args,
                  out_specs=(spec,) * len(out_names), check_rep=False),
        keep_unused=True,
    )
    return f, in_names, out_names, out_avals, NamedSharding(mesh, spec)


def _run_cached(X, tkeep, fkeep):
    import jax

    if "fn" not in _BASS_CACHE:
        _BASS_CACHE["fn"] = _build_cached_callable(_get_bass())
    f, in_names, out_names, out_avals, sharding = _BASS_CACHE["fn"]
    arrays = {"x": X, "tk": tkeep, "fk": fkeep}
    concat_in = [jax.device_put(arrays[n], sharding) for n in in_names]
    concat_zeros = [
        jax.device_put(np.zeros((NCORES * a.shape[0], *a.shape[1:]), a.dtype), sharding)
        for a in out_avals
    ]
    outs = f(*concat_in, *concat_zeros)
    return np.asarray(outs[out_names.index("out")]).reshape(B, T, D)


def _run_on_device(X, tkeep, fkeep, trace=False):
    from concourse.bass_utils import run_bass_kernel_spmd

    nc = _get_bass()
    X = np.ascontiguousarray(np.asarray(X, dtype=np.float32))
    in_maps = []
    for i in range(NCORES):
        sl = slice(i * BPC, (i + 1) * BPC)
        in_maps.append({
            "x": np.ascontiguousarray(X[sl]),
            "tk": np.ascontiguousarray(tkeep[sl]),
            "fk": np.ascontiguousarray(fkeep[sl]),
        })
    res = run_bass_kernel_spmd(nc, in_maps, core_ids=list(range(NCORES)), trace=trace)
    out = np.concatenate([r["out"] for r in res.results], axis=0)
    return out, res


def kernel(X, lengths):
    tkeep, fkeep = _host_masks(lengths)
    X = np.ascontiguousarray(np.asarray(X, dtype=np.float32))
    if _BASS_CACHE.get("warm"):
        # repeat call: reuse the cached jitted executable (no re-lowering)
        try:
            return _run_cached(X, tkeep, fkeep)
        except Exception:
            pass
    out, _ = _run_on_device(X, tkeep, fkeep, trace=False)
    _BASS_CACHE["warm"] = True
    return out
